# revision 2
# baseline (speedup 1.0000x reference)
"""Causal single-head attention on 8 trn2 NeuronCores (Bass/Tile).

Problem: X [4, 2048, 2048] f32, W_Q/W_K/W_V [2048, 256] f32.
  Z = softmax(mask((X@W_Q)(X@W_K)^T / sqrt(256))) @ (X@W_V)

Sharding: 8 cores = 4 batches x 2 query-stripes. Core (b, s) handles the
queries of batch b at token positions == s (mod 2) -- striping balances the
causal attention work exactly across the two cores of a batch, and makes the
per-core mask structure uniform (the only cross-core difference is whether
the remote stripe's same-index key is visible, which is folded into a tiny
per-core [128,128] additive triangle-mask input).

On-chip layout: the host pre-transposes X to feature-major XT [d_model, rows]
(bf16, partition-major so every DMA is contiguous per partition), so every
projection is a natural PE matmul (contraction on the partition axis).
Attention scores are computed transposed, S^T = K Q^T in [keys, queries]
tiles; exp runs on ScalarE straight out of PSUM (no max subtraction -- score
scale here is ~N(0, 1.8), exp stays well inside f32 range); the softmax
denominator falls out of the P^T @ V_aug matmul via a ones-column appended to
V. Diagonal-band tiles only compute the visible query range, and only the
single triangular 128x128 block gets an additive mask (host-provided input).
All matmuls are bf16; accumulation is f32 in PSUM; final normalize is f32.

Each core projects K/V only for its own stripe; the other stripe's K/V
arrives via two pairwise 2-rank AllGathers (K^T right after the K
projection, V after the V projection).  ncfw cannot move collective data
before its ~35us init wall and serializes same-chip collectives; the whole
schedule is built to (a) trigger the K AG as early as possible (input DMAs
split across the sync+act HWDGE queues in K-consumption order so the K
projection is DMA-paced to ~<20us, bounces ride sync which is free by then)
and (b) keep the PE busy with V-independent work (V/Q projections, seg0
attention, seg1 scores) until the V AllGather lands.  The seg1 AV tail is
minimized by preloading each output PSUM bank with the stashed seg0 partial
(vector copy, off the critical path, start=False accumulation) so the only
post-AV work per 128-query block is reciprocal + scalar-mul + out-DMA.

Measured (2026-08-08, old schedule): 93.3-104.3us.  Exchange cost model
(hardware-measured): AG(size) = max(floor 7-19us, size/~33-70GB/s),
serialized on one ncfw stream behind a ~35us init wall; AG data starts
~7us after its bounce lands.  Closed dead ends (do not retry on this
runtime): merged >1MiB collective (RDH crash at 1MiB+4KB), merged 897KB
single AG (bandwidth-bound, 112us), half-dup with ~256KB AGs + local
recompute (floors don't shrink below ~512KB; 105us), remote_dma
(unsupported, wedges terminal), AllToAll (2-rank unsupported), 8-rank V
gather (no stream parallelism, 138us), fp8 anywhere (score/P/V element
errors do NOT average down in Z -- rel err scales with the element error
itself; measured 2.8e-2 for fp8 K alone vs the 2e-2 gate), warmup
collective (adds queue delay), column-split XT loads (halve DMA
efficiency).  Exact local recompute of remote K/V halves from extra
remote-X input columns WORKS (bit-identical) -- it just doesn't pay at
this problem size (PE becomes the bottleneck at ~71us of matmul issue).

kernel() takes the FULL inputs and returns the FULL output.
"""

from contextlib import ExitStack

import numpy as np
import ml_dtypes

import concourse.bass as bass
import concourse.tile as tile
from concourse import bacc, mybir
from concourse.bass_utils import run_bass_kernel_spmd

BF16 = mybir.dt.bfloat16
F32 = mybir.dt.float32

B, L, D, DK, DV = 4, 2048, 2048, 256, 256
LQ = L // 2          # queries per core (one stripe)
NT = D // 128        # 16 d_model tiles
KSEG = LQ // 128     # 8 key tiles per segment
CHUNK = 512          # query free-dim chunk for the scores matmul
NCHUNK = LQ // CHUNK
Q4 = CHUNK // 128     # query subtiles per chunk
SCALE = 1.0 / float(np.sqrt(DK))
MASK = -1e9

MODE = "coll2"


def build_kernel(mode: str = "coll2"):
    nc = bacc.Bacc("TRN2", target_bir_lowering=False, debug=False, num_devices=8)

    xt_ext = nc.declare_dram_parameter("XT", [128, NT, LQ], BF16, isOutput=False)
    wq_ext = nc.declare_dram_parameter("WQ", [128, NT, DK], BF16, isOutput=False)
    wk_ext = nc.declare_dram_parameter("WK", [128, NT, DK], BF16, isOutput=False)
    wv_ext = nc.declare_dram_parameter("WV", [128, NT, DV], BF16, isOutput=False)
    # trimask[:, seg, :]: additive f32 [128 keys, 128 queries] triangle mask
    # for the diagonal block of the diagonal-band score tiles.
    trimask_ext = nc.declare_dram_parameter("TRIMASK", [128, 2, 128], F32, isOutput=False)
    # conds[0, i] == 1 iff the pair peer's AllGather slot is i (i.e. 1-s == i)
    conds_ext = nc.declare_dram_parameter("CONDS", [1, 2], mybir.dt.uint32, isOutput=False)
    out_ext = nc.declare_dram_parameter("OUT", [LQ, DV], F32, isOutput=True)

    # DRAM bounce buffers for the pairwise K/V exchange (bf16).
    vcols = KSEG * (DV + 1)  # 2056
    kt_bounce = nc.dram_tensor("kt_bounce", [128, 2 * LQ], BF16)
    kt_gat = nc.dram_tensor("kt_gat", [2, 128, 2 * LQ], BF16)
    v_bounce = nc.dram_tensor("v_bounce", [128, vcols], BF16)
    v_gat = nc.dram_tensor("v_gat", [2, 128, vcols], BF16)

    with tile.TileContext(nc) as tc, ExitStack() as ctx:
        const = ctx.enter_context(tc.tile_pool(name="const", bufs=1))
        xt_pool = ctx.enter_context(tc.tile_pool(name="xt", bufs=1))
        # One PSUM pool, two tags: "s" (proj/score chains, 2 banks
        # double-buffered) and "o" (output accumulators, 6 banks so up to six
        # 128-query blocks accumulate concurrently).  2 + 6 = all 8 banks.
        psum = ctx.enter_context(tc.tile_pool(name="psum", bufs=2, space="PSUM"))
        ptile_pool = ctx.enter_context(tc.tile_pool(name="ptile", bufs=6))
        small = ctx.enter_context(tc.tile_pool(name="small", bufs=4))

        rg = [[0, 1], [2, 3], [4, 5], [6, 7]]

        # ---- input DMAs, split across the two HWDGE queues ----------------
        # sync: xt dtile groups 0..11 (3.15MB).  act: wk, xt groups 12..15,
        # wv, wq, trimask (4.2MB).  Everything the K projection needs lands
        # first on both queues; the K/V bounce DMAs ride sync, which is idle
        # again by the time the K projection finishes.
        wk = const.tile([128, NT, DK], BF16)
        wv = const.tile([128, NT, DV], BF16)
        wq = const.tile([128, NT, DK], BF16)
        xt_sb = xt_pool.tile([128, NT, LQ], BF16, name="xt_sb")
        xt = [xt_sb[:, dt, :] for dt in range(NT)]
        trimask = const.tile([128, 2, 128], F32)

        nc.scalar.dma_start(wk[:], wk_ext.ap())
        nc.sync.dma_start(xt_sb[:, 0:4, :], xt_ext[:, 0:4, :])
        nc.scalar.dma_start(xt_sb[:, 12:16, :], xt_ext[:, 12:16, :])
        nc.sync.dma_start(xt_sb[:, 4:8, :], xt_ext[:, 4:8, :])
        nc.scalar.dma_start(wv[:], wv_ext.ap())
        nc.sync.dma_start(xt_sb[:, 8:12, :], xt_ext[:, 8:12, :])
        nc.scalar.dma_start(wq[:], wq_ext.ap())
        nc.scalar.dma_start(trimask[:], trimask_ext.ap())

        # ---- K^T projection (local stripe): [128, 2(m), LQ] bf16 ----------
        kt_loc = const.tile([128, 2, LQ], BF16)
        kt_rem = const.tile([128, 2, LQ], BF16)
        kt_all = [kt_loc, kt_rem]

        def project_kt_chain(dst, w, m, n):
            col0 = n * CHUNK
            ps = psum.tile([128, CHUNK], F32, name="ps", tag="s")
            for dt in range(NT):
                nc.tensor.matmul(
                    ps[:],
                    w[:, dt, m * 128:(m + 1) * 128],
                    xt[dt][:, col0:col0 + CHUNK],
                    start=(dt == 0),
                    stop=(dt == NT - 1),
                )
            nc.scalar.copy(dst[:, m, n * CHUNK:(n + 1) * CHUNK], ps[:])

        for n in range(LQ // CHUNK):
            for m in range(2):
                project_kt_chain(kt_loc, wk, m, n)

        # K exchange: bounce to DRAM (sync queue) + pairwise AllGather.
        nc.sync.dma_start(kt_bounce[:, :], kt_loc.rearrange("p m q -> p (m q)"))
        nc.gpsimd.collective_compute(
            "AllGather", mybir.AluOpType.bypass, replica_groups=rg,
            ins=[kt_bounce.ap()], outs=[kt_gat.ap()],
        )

        # ---- V projection (local stripe): [128, KSEG, 257] bf16 ------------
        v_loc = const.tile([128, KSEG, DV + 1], BF16)
        v_rem = const.tile([128, KSEG, DV + 1], BF16)
        v_all = [v_loc, v_rem]

        nc.vector.memset(v_loc[:, :, DV:DV + 1], 1.0)
        for rt in range(KSEG):
            ps = psum.tile([128, DV], F32, name="ps", tag="s")
            for dt in range(NT):
                nc.tensor.matmul(
                    ps[:],
                    xt[dt][:, rt * 128:(rt + 1) * 128],
                    wv[:, dt, :],
                    start=(dt == 0),
                    stop=(dt == NT - 1),
                )
            nc.scalar.copy(v_loc[:, rt, 0:DV], ps[:])

        # V exchange.
        nc.sync.dma_start(v_bounce[:, :], v_loc.rearrange("p t c -> p (t c)"))
        nc.gpsimd.collective_compute(
            "AllGather", mybir.AluOpType.bypass, replica_groups=rg,
            ins=[v_bounce.ap()], outs=[v_gat.ap()],
        )

        # ---- fetch the pair peer's gathered K^T and V directly ------------
        # Two conditional DMAs per tensor: per-core CONDS input decides which
        # AllGather slot is the peer's; the other DMA is skipped entirely
        # (cond -> skip_entire_dma; the skipped DMA still bumps semaphores,
        # keeping Tile's dependency bookkeeping intact).
        cond_regs = []
        for i in range(2):
            r = nc.sync.alloc_register(f"peer_cond_{i}")
            nc.sync.reg_load(r, conds_ext[0:1, i:i + 1])
            cond_regs.append(
                nc.sync.snap(r, donate=True, min_val=0, max_val=1)
            )
        kt_rem_flat = kt_rem.rearrange("p m q -> p (m q)")
        v_rem_flat = v_rem.rearrange("p t c -> p (t c)")

        # ---- Q^T projection + seg0 attention, interleaved per chunk --------
        qt = const.tile([128, 2, LQ], BF16)

        def scores_exp(c, seg, kb, p_out):
            j = kb - Q4 * c
            if j > 0:
                cols = CHUNK - j * 128
                q0 = c * CHUNK + j * 128
                o0 = j * 128
            else:
                cols = CHUNK
                q0 = c * CHUNK
                o0 = 0
            s_ps = psum.tile([128, cols], F32, name="ps", tag="s")
            for m in range(2):
                nc.tensor.matmul(
                    s_ps[:],
                    kt_all[seg][:, m, kb * 128:(kb + 1) * 128],
                    qt[:, m, q0:q0 + cols],
                    start=(m == 0),
                    stop=(m == 1),
                )
            if j >= 0:
                # diagonal block = first 128 cols of the (trimmed) range
                nc.vector.tensor_add(
                    s_ps[:, 0:128], s_ps[:, 0:128], trimask[:, seg, :]
                )
            nc.scalar.activation(
                p_out[:, o0:o0 + cols], s_ps[:],
                mybir.ActivationFunctionType.Exp, scale=SCALE,
            )

        def av(c, seg, kb, p, o_ps, start):
            for q in range(Q4):
                ti = Q4 * c + q
                if kb > ti:
                    continue
                nc.tensor.matmul(
                    o_ps[q][:],
                    p[:, q * 128:(q + 1) * 128],
                    v_all[seg][:, kb, :],
                    start=start and (kb == 0),
                    stop=(kb == ti),
                )

        # Local-segment attention, software-pipelined so the PE is never
        # waiting on the exp of the tile it is about to consume: scores run
        # LOOK tiles ahead of the AV accumulations.  Each chunk's Q
        # projection is emitted right before it, so seg0 chunk 0 starts as
        # soon as the first half of Q^T exists.
        o_stash = {}
        LOOK = 2

        def seg0_chunk(c):
            o_ps = [
                psum.tile([128, DV + 1], F32, name="o_ps", tag="o", bufs=6)
                for _ in range(Q4)
            ]
            kbmax = Q4 * (c + 1)
            plist = {}

            def do_av(kb):
                av(c, 0, kb, plist[kb], o_ps, start=True)
                for q in range(Q4):
                    if kb == Q4 * c + q:  # accumulation for q just stopped
                        st = const.tile(
                            [128, DV + 1], F32, name=f"o_stash_{c}_{q}"
                        )
                        o_stash[(c, q)] = st
                        nc.vector.tensor_copy(st[:], o_ps[q][:])

            for kb in range(kbmax):
                p = ptile_pool.tile([128, CHUNK], BF16, name="p")
                plist[kb] = p
                scores_exp(c, 0, kb, p)
                if kb >= LOOK:
                    do_av(kb - LOOK)
            for kb in range(max(0, kbmax - LOOK), kbmax):
                do_av(kb)

        for c in range(NCHUNK):
            for m in range(2):
                ps = psum.tile([128, CHUNK], F32, name="ps", tag="s")
                for dt in range(NT):
                    nc.tensor.matmul(
                        ps[:],
                        wq[:, dt, m * 128:(m + 1) * 128],
                        xt[dt][:, c * CHUNK:(c + 1) * CHUNK],
                        start=(dt == 0),
                        stop=(dt == NT - 1),
                    )
                nc.scalar.copy(qt[:, m, c * CHUNK:(c + 1) * CHUNK], ps[:])
            seg0_chunk(c)

        # Pull the peer's K^T as soon as its AllGather lands (sync queue is
        # idle by now), then compute all remote-segment scores (V-independent)
        nc.sync.dma_start(kt_rem_flat[:], kt_gat[0, :, :], cond=cond_regs[0])
        nc.sync.dma_start(kt_rem_flat[:], kt_gat[1, :, :], cond=cond_regs[1])
        p_store = {}
        for c in range(NCHUNK):
            for kb in range(Q4 * (c + 1)):
                t = const.tile([128, CHUNK], BF16, name=f"p_rem_{c}_{kb}")
                p_store[(c, kb)] = t
                scores_exp(c, 1, kb, t)

        nc.sync.dma_start(v_rem_flat[:], v_gat[0, :, :], cond=cond_regs[0])
        nc.sync.dma_start(v_rem_flat[:], v_gat[1, :, :], cond=cond_regs[1])

        # Remote-segment AV, one accumulation chain per 128-query block:
        # preload the PSUM bank with the stashed seg0 partial sums (vector
        # copy, runs while the V AllGather is still in flight), accumulate
        # seg1 with start=False, then the only tail work per block is
        # reciprocal + scalar-mul + out-DMA.  Preloads for the last two
        # blocks are emitted after earlier normalizes so the 6-slot "o" ring
        # never deadlocks the vector queue on itself.
        o_seg1 = {}

        def preload(ti):
            c, q = ti // Q4, ti % Q4
            o_ps = psum.tile([128, DV + 1], F32, name="o_ps", tag="o", bufs=6)
            o_seg1[ti] = o_ps
            nc.vector.tensor_copy(o_ps[:], o_stash[(c, q)][:])

        for ti in range(6):
            preload(ti)
        for ti in range(2 * Q4):
            c, q = ti // Q4, ti % Q4
            o_ps = o_seg1[ti]
            for kb in range(ti + 1):
                nc.tensor.matmul(
                    o_ps[:],
                    p_store[(c, kb)][:, q * 128:(q + 1) * 128],
                    v_all[1][:, kb, :],
                    start=False,
                    stop=(kb == ti),
                )
            recip = small.tile([128, 1], F32, name="recip")
            nc.vector.reciprocal(recip[:], o_ps[:, DV:DV + 1])
            o_sb = small.tile([128, DV], F32, name="o_sb")
            nc.vector.tensor_scalar_mul(o_sb[:], o_ps[:, 0:DV], recip[:])
            nc.sync.dma_start(out_ext[ti * 128:(ti + 1) * 128, :], o_sb[:])
            if ti + 6 < 2 * Q4:
                preload(ti + 6)

    nc.finalize()
    return nc


_CACHED = {}


def _get_kernel(mode: str):
    if mode not in _CACHED:
        _CACHED[mode] = build_kernel(mode)
    return _CACHED[mode]


def _prepare_in_maps(X, W_Q, W_K, W_V, mode):
    def wlayout(W):
        # w[p, dt, c] = W[dt*128 + p, c]
        n = W.shape[1]
        return np.ascontiguousarray(
            W.reshape(NT, 128, n).transpose(1, 0, 2)
        ).astype(ml_dtypes.bfloat16)

    wq = wlayout(W_Q)
    wk = wlayout(W_K)
    wv = wlayout(W_V)

    # Triangle masks for the diagonal 128x128 block of diagonal-band tiles.
    # Element [p, x]: key-in-block p, query-in-block x.
    # seg0 (own stripe): visible iff p <= x.
    # seg1 (remote stripe): s=0 cores: visible iff p < x; s=1: p <= x.
    p_idx = np.arange(128)[:, None]
    x_idx = np.arange(128)[None, :]
    tri_incl = np.where(p_idx <= x_idx, 0.0, MASK).astype(np.float32)
    tri_excl = np.where(p_idx < x_idx, 0.0, MASK).astype(np.float32)

    in_maps = []
    for core in range(8):
        b, s = core // 2, core % 2
        # partition-major layout: xt[p, dt, r] = X[b, stripe r, dt*128 + p]
        loc = X[b, s::2, :].reshape(LQ, NT, 128).transpose(2, 1, 0)
        xt = np.ascontiguousarray(loc).astype(ml_dtypes.bfloat16)
        trimask = np.stack(
            [tri_incl, tri_excl if s == 0 else tri_incl], axis=1
        )  # [128, 2, 128]
        conds = np.zeros((1, 2), np.uint32)
        conds[0, 1 - s] = 1  # the pair peer's slot in the gather
        in_maps.append(
            {"XT": xt, "WQ": wq, "WK": wk, "WV": wv,
             "TRIMASK": np.ascontiguousarray(trimask), "CONDS": conds}
        )
    return in_maps


def _assemble(results):
    Z = np.empty((B, L, DV), np.float32)
    for core in range(8):
        b, s = core // 2, core % 2
        Z[b, s::2, :] = results[core]["OUT"]
    return Z


def kernel(X, W_Q, W_K, W_V):
    nc = _get_kernel(MODE)
    in_maps = _prepare_in_maps(X, W_Q, W_K, W_V, MODE)
    res = run_bass_kernel_spmd(nc, in_maps, core_ids=list(range(8)))
    return _assemble(res.results)


# revision 15
# speedup vs baseline: 1.0227x; 1.0227x over previous
"""Causal single-head attention on 8 trn2 NeuronCores (Bass/Tile).

Problem: X [4, 2048, 2048] f32, W_Q/W_K/W_V [2048, 256] f32.
  Z = softmax(mask((X@W_Q)(X@W_K)^T / sqrt(256))) @ (X@W_V)

Sharding: 8 cores = 4 batches x 2 query-stripes. Core (b, s) handles the
queries of batch b at token positions == s (mod 2) -- striping balances the
causal attention work exactly across the two cores of a batch, and makes the
per-core mask structure uniform (the only cross-core difference is whether
the remote stripe's same-index key is visible, which is folded into a tiny
per-core [128,128] additive triangle-mask input).

On-chip layout: the host pre-transposes X to feature-major XT [d_model, rows]
(bf16, partition-major so every DMA is contiguous per partition), so every
projection is a natural PE matmul (contraction on the partition axis).
Attention scores are computed transposed, S^T = K Q^T in [keys, queries]
tiles; exp runs on ScalarE straight out of PSUM (no max subtraction -- score
scale here is ~N(0, 1.8), exp stays well inside f32 range); the softmax
denominator falls out of the P^T @ V_aug matmul via a ones-column appended to
V. Diagonal-band tiles only compute the visible query range, and only the
single triangular 128x128 block gets an additive mask (host-provided input).
All matmuls are bf16; accumulation is f32 in PSUM; final normalize is f32.

Each core projects K/V only for its own stripe; the other stripe's K/V
arrives via two pairwise 2-rank AllGathers (K^T right after the K
projection, V after the V projection).  ncfw cannot move collective data
before its ~35us init wall and serializes same-chip collectives; the whole
schedule is built to (a) trigger the K AG as early as possible (input DMAs
split across the sync+act HWDGE queues in K-consumption order so the K
projection is DMA-paced to ~<20us, bounces ride sync which is free by then)
and (b) keep the PE busy with V-independent work (V/Q projections, seg0
attention, seg1 scores) until the V AllGather lands.  The seg1 AV tail is
minimized by preloading each output PSUM bank with the stashed seg0 partial
(vector copy, off the critical path, start=False accumulation) so the only
post-AV work per 128-query block is reciprocal + scalar-mul + out-DMA.

Measured (2026-08-08, old schedule): 93.3-104.3us.  Exchange cost model
(hardware-measured): AG(size) = max(floor 7-19us, size/~33-70GB/s),
serialized on one ncfw stream behind a ~35us init wall; AG data starts
~7us after its bounce lands.  Closed dead ends (do not retry on this
runtime): merged >1MiB collective (RDH crash at 1MiB+4KB), merged 897KB
single AG (bandwidth-bound, 112us), half-dup with ~256KB AGs + local
recompute (floors don't shrink below ~512KB; 105us), remote_dma
(unsupported, wedges terminal), AllToAll (2-rank unsupported), 8-rank V
gather (no stream parallelism, 138us), fp8 anywhere (score/P/V element
errors do NOT average down in Z -- rel err scales with the element error
itself; measured 2.8e-2 for fp8 K alone vs the 2e-2 gate), warmup
collective (adds queue delay), column-split XT loads (halve DMA
efficiency).  Exact local recompute of remote K/V halves from extra
remote-X input columns WORKS (bit-identical) -- it just doesn't pay at
this problem size (PE becomes the bottleneck at ~71us of matmul issue).

kernel() takes the FULL inputs and returns the FULL output.
"""

from contextlib import ExitStack

import numpy as np
import ml_dtypes

import concourse.bass as bass
import concourse.tile as tile
from concourse import bacc, mybir
from concourse.bass_utils import run_bass_kernel_spmd

BF16 = mybir.dt.bfloat16
F32 = mybir.dt.float32

B, L, D, DK, DV = 4, 2048, 2048, 256, 256
LQ = L // 2          # queries per core (one stripe)
NT = D // 128        # 16 d_model tiles
KSEG = LQ // 128     # 8 key tiles per segment
CHUNK = 512          # query free-dim chunk for the scores matmul
NCHUNK = LQ // CHUNK
Q4 = CHUNK // 128     # query subtiles per chunk
SCALE = 1.0 / float(np.sqrt(DK))
MASK = -1e9

MODE = "coll2"


def build_kernel(mode: str = "coll2"):
    nc = bacc.Bacc("TRN2", target_bir_lowering=False, debug=False, num_devices=8)

    # XT[p, h, dt, c]: h = 512-token column half of this core's stripe.
    # Column-half-major so the whole of half 0 (everything the first K/Q
    # projection chains touch) streams in one contiguous run per partition,
    # letting the dt-interleaved projection pairs track the DMA tile by tile.
    xt_ext = nc.declare_dram_parameter("XT", [128, 2, NT, CHUNK], BF16, isOutput=False)
    wq_ext = nc.declare_dram_parameter("WQ", [128, NT, DK], BF16, isOutput=False)
    wk_ext = nc.declare_dram_parameter("WK", [128, NT, DK], BF16, isOutput=False)
    wv_ext = nc.declare_dram_parameter("WV", [128, NT, DV], BF16, isOutput=False)
    # trimask[:, seg, :]: additive f32 [128 keys, 128 queries] triangle mask
    # for the diagonal block of the diagonal-band score tiles.
    trimask_ext = nc.declare_dram_parameter("TRIMASK", [128, 2, 128], F32, isOutput=False)
    # conds[0, i] == 1 iff the pair peer's AllGather slot is i (i.e. 1-s == i)
    conds_ext = nc.declare_dram_parameter("CONDS", [1, 2], mybir.dt.uint32, isOutput=False)
    out_ext = nc.declare_dram_parameter("OUT", [LQ, DV], F32, isOutput=True)

    # DRAM bounce buffers for the pairwise K/V exchange (bf16).
    vcols = KSEG * (DV + 1)  # 2056
    kt_bounce = nc.dram_tensor("kt_bounce", [128, 2 * LQ], BF16)
    kt_gat = nc.dram_tensor("kt_gat", [2, 128, 2 * LQ], BF16)
    v_bounce = nc.dram_tensor("v_bounce", [128, vcols], BF16)
    v_gat = nc.dram_tensor("v_gat", [2, 128, vcols], BF16)

    with tile.TileContext(nc) as tc, ExitStack() as ctx:
        const = ctx.enter_context(tc.tile_pool(name="const", bufs=1))
        xt_pool = ctx.enter_context(tc.tile_pool(name="xt", bufs=1))
        # One PSUM pool, two tags: "s" (proj/score chains, 2 banks
        # double-buffered) and "o" (output accumulators, 6 banks so up to six
        # 128-query blocks accumulate concurrently).  2 + 6 = all 8 banks.
        psum = ctx.enter_context(tc.tile_pool(name="psum", bufs=2, space="PSUM"))
        ptile_pool = ctx.enter_context(tc.tile_pool(name="ptile", bufs=6))
        small = ctx.enter_context(tc.tile_pool(name="small", bufs=4))

        rg = [[0, 1], [2, 3], [4, 5], [6, 7]]

        # ---- input DMAs, split across the two HWDGE queues ----------------
        # act (the faster-ramping queue) carries the X stripe, column half 0
        # then half 1, in exactly the order the dt-interleaved K projection
        # pairs consume it; sync carries the weights (small first chunk so
        # the PE's first LDWEIGHTS fires early) + trimask, and is free again
        # by the time the K/V bounces need it.
        wk = const.tile([128, NT, DK], BF16)
        wv = const.tile([128, NT, DV], BF16)
        wq = const.tile([128, NT, DK], BF16)
        xt_sb = xt_pool.tile([128, 2, NT, CHUNK], BF16, name="xt_sb")
        xt = [[xt_sb[:, h, dt, :] for dt in range(NT)] for h in range(2)]
        trimask = const.tile([128, 2, 128], F32)

        nc.sync.dma_start(wk[:, 0:4, :], wk_ext[:, 0:4, :])
        nc.scalar.dma_start(xt_sb[:, 0, 0:4, :], xt_ext[:, 0, 0:4, :])
        nc.sync.dma_start(wk[:, 4:16, :], wk_ext[:, 4:16, :])
        nc.scalar.dma_start(xt_sb[:, 0, 4:10, :], xt_ext[:, 0, 4:10, :])
        nc.scalar.dma_start(xt_sb[:, 0, 10:16, :], xt_ext[:, 0, 10:16, :])
        nc.sync.dma_start(wv[:], wv_ext.ap())
        nc.scalar.dma_start(xt_sb[:, 1, 0:6, :], xt_ext[:, 1, 0:6, :])
        nc.sync.dma_start(wq[:], wq_ext.ap())
        nc.scalar.dma_start(xt_sb[:, 1, 6:11, :], xt_ext[:, 1, 6:11, :])
        nc.scalar.dma_start(xt_sb[:, 1, 11:16, :], xt_ext[:, 1, 11:16, :])
        nc.sync.dma_start(trimask[:], trimask_ext.ap())

        # ---- K^T projection (local stripe): [128, 2(m), LQ] bf16 ----------
        # Per column half, both dk-half accumulation chains run dt-
        # interleaved in two PSUM banks, so each arriving X d-tile is
        # consumed immediately (PE tracks the DMA instead of idling until
        # the whole stripe lands).
        kt_loc = const.tile([128, 2, LQ], BF16)
        kt_rem = const.tile([128, 2, LQ], BF16)
        kt_all = [kt_loc, kt_rem]

        for n in range(LQ // CHUNK):
            ps0 = psum.tile([128, CHUNK], F32, name="ps0", tag="s")
            ps1 = psum.tile([128, CHUNK], F32, name="ps1", tag="s")
            for dt in range(NT):
                nc.tensor.matmul(
                    ps0[:], wk[:, dt, 0:128], xt[n][dt][:],
                    start=(dt == 0), stop=(dt == NT - 1),
                )
                nc.tensor.matmul(
                    ps1[:], wk[:, dt, 128:256], xt[n][dt][:],
                    start=(dt == 0), stop=(dt == NT - 1),
                )
            nc.scalar.copy(kt_loc[:, 0, n * CHUNK:(n + 1) * CHUNK], ps0[:])
            nc.scalar.copy(kt_loc[:, 1, n * CHUNK:(n + 1) * CHUNK], ps1[:])

        # K exchange: bounce to DRAM (sync queue) + pairwise AllGather.
        nc.sync.dma_start(kt_bounce[:, :], kt_loc.rearrange("p m q -> p (m q)"))
        nc.gpsimd.collective_compute(
            "AllGather", mybir.AluOpType.bypass, replica_groups=rg,
            ins=[kt_bounce.ap()], outs=[kt_gat.ap()],
        )

        # ---- V projection (local stripe): [128, KSEG, 257] bf16 ------------
        v_loc = const.tile([128, KSEG, DV + 1], BF16)
        v_rem = const.tile([128, KSEG, DV + 1], BF16)
        v_all = [v_loc, v_rem]

        nc.vector.memset(v_loc[:, :, DV:DV + 1], 1.0)
        for rt in range(KSEG):
            ps = psum.tile([128, DV], F32, name="ps", tag="s")
            h, r = rt // 4, rt % 4
            for dt in range(NT):
                nc.tensor.matmul(
                    ps[:],
                    xt[h][dt][:, r * 128:(r + 1) * 128],
                    wv[:, dt, :],
                    start=(dt == 0),
                    stop=(dt == NT - 1),
                )
            nc.scalar.copy(v_loc[:, rt, 0:DV], ps[:])

        # V exchange.
        nc.sync.dma_start(v_bounce[:, :], v_loc.rearrange("p t c -> p (t c)"))
        nc.gpsimd.collective_compute(
            "AllGather", mybir.AluOpType.bypass, replica_groups=rg,
            ins=[v_bounce.ap()], outs=[v_gat.ap()],
        )

        # ---- fetch the pair peer's gathered K^T and V directly ------------
        # Two conditional DMAs per tensor: per-core CONDS input decides which
        # AllGather slot is the peer's; the other DMA is skipped entirely
        # (cond -> skip_entire_dma; the skipped DMA still bumps semaphores,
        # keeping Tile's dependency bookkeeping intact).
        cond_regs = []
        for i in range(2):
            r = nc.sync.alloc_register(f"peer_cond_{i}")
            nc.sync.reg_load(r, conds_ext[0:1, i:i + 1])
            cond_regs.append(
                nc.sync.snap(r, donate=True, min_val=0, max_val=1)
            )
        kt_rem_flat = kt_rem.rearrange("p m q -> p (m q)")
        v_rem_flat = v_rem.rearrange("p t c -> p (t c)")

        # ---- Q^T projection + seg0 attention, interleaved per chunk --------
        qt = const.tile([128, 2, LQ], BF16)

        def scores_exp(c, seg, kb, p_out):
            j = kb - Q4 * c
            if j > 0:
                cols = CHUNK - j * 128
                q0 = c * CHUNK + j * 128
                o0 = j * 128
            else:
                cols = CHUNK
                q0 = c * CHUNK
                o0 = 0
            s_ps = psum.tile([128, cols], F32, name="ps", tag="s")
            for m in range(2):
                nc.tensor.matmul(
                    s_ps[:],
                    kt_all[seg][:, m, kb * 128:(kb + 1) * 128],
                    qt[:, m, q0:q0 + cols],
                    start=(m == 0),
                    stop=(m == 1),
                )
            if j >= 0:
                # diagonal block = first 128 cols of the (trimmed) range
                nc.vector.tensor_add(
                    s_ps[:, 0:128], s_ps[:, 0:128], trimask[:, seg, :]
                )
            nc.scalar.activation(
                p_out[:, o0:o0 + cols], s_ps[:],
                mybir.ActivationFunctionType.Exp, scale=SCALE,
            )

        def av(c, seg, kb, p, o_ps, start):
            for q in range(Q4):
                ti = Q4 * c + q
                if kb > ti:
                    continue
                nc.tensor.matmul(
                    o_ps[q][:],
                    p[:, q * 128:(q + 1) * 128],
                    v_all[seg][:, kb, :],
                    start=start and (kb == 0),
                    stop=(kb == ti),
                )

        # Local-segment attention, software-pipelined so the PE is never
        # waiting on the exp of the tile it is about to consume: scores run
        # LOOK tiles ahead of the AV accumulations.  Each chunk's Q
        # projection is emitted right before it, so seg0 chunk 0 starts as
        # soon as the first half of Q^T exists.
        o_stash = {}
        LOOK = 2

        def seg0_chunk(c):
            o_ps = [
                psum.tile([128, DV + 1], F32, name="o_ps", tag="o", bufs=6)
                for _ in range(Q4)
            ]
            kbmax = Q4 * (c + 1)
            plist = {}

            def do_av(kb):
                av(c, 0, kb, plist[kb], o_ps, start=True)
                for q in range(Q4):
                    if kb == Q4 * c + q:  # accumulation for q just stopped
                        st = const.tile(
                            [128, DV + 1], F32, name=f"o_stash_{c}_{q}"
                        )
                        o_stash[(c, q)] = st
                        nc.vector.tensor_copy(st[:], o_ps[q][:])

            for kb in range(kbmax):
                p = ptile_pool.tile([128, CHUNK], BF16, name="p")
                plist[kb] = p
                scores_exp(c, 0, kb, p)
                if kb >= LOOK:
                    do_av(kb - LOOK)
            for kb in range(max(0, kbmax - LOOK), kbmax):
                do_av(kb)

        for c in range(NCHUNK):
            for m in range(2):
                ps = psum.tile([128, CHUNK], F32, name="ps", tag="s")
                for dt in range(NT):
                    nc.tensor.matmul(
                        ps[:],
                        wq[:, dt, m * 128:(m + 1) * 128],
                        xt[c][dt][:],
                        start=(dt == 0),
                        stop=(dt == NT - 1),
                    )
                nc.scalar.copy(qt[:, m, c * CHUNK:(c + 1) * CHUNK], ps[:])
            seg0_chunk(c)

        # Pull the peer's K^T as soon as its AllGather lands (sync queue is
        # idle by now), then compute all remote-segment scores (V-independent)
        nc.sync.dma_start(kt_rem_flat[:], kt_gat[0, :, :], cond=cond_regs[0])
        nc.sync.dma_start(kt_rem_flat[:], kt_gat[1, :, :], cond=cond_regs[1])
        p_store = {}
        for c in range(NCHUNK):
            for kb in range(Q4 * (c + 1)):
                t = const.tile([128, CHUNK], BF16, name=f"p_rem_{c}_{kb}")
                p_store[(c, kb)] = t
                scores_exp(c, 1, kb, t)

        nc.sync.dma_start(v_rem_flat[:], v_gat[0, :, :], cond=cond_regs[0])
        nc.sync.dma_start(v_rem_flat[:], v_gat[1, :, :], cond=cond_regs[1])

        # Remote-segment AV, one accumulation chain per 128-query block:
        # preload the PSUM bank with the stashed seg0 partial sums (vector
        # copy, runs while the V AllGather is still in flight), accumulate
        # seg1 with start=False, then the only tail work per block is
        # reciprocal + scalar-mul + out-DMA.  Preloads for the last two
        # blocks are emitted after earlier normalizes so the 6-slot "o" ring
        # never deadlocks the vector queue on itself.
        o_seg1 = {}

        def preload(ti):
            c, q = ti // Q4, ti % Q4
            o_ps = psum.tile([128, DV + 1], F32, name="o_ps", tag="o", bufs=6)
            o_seg1[ti] = o_ps
            nc.vector.tensor_copy(o_ps[:], o_stash[(c, q)][:])

        for ti in range(6):
            preload(ti)
        for ti in range(2 * Q4):
            c, q = ti // Q4, ti % Q4
            o_ps = o_seg1[ti]
            for kb in range(ti + 1):
                nc.tensor.matmul(
                    o_ps[:],
                    p_store[(c, kb)][:, q * 128:(q + 1) * 128],
                    v_all[1][:, kb, :],
                    start=False,
                    stop=(kb == ti),
                )
            recip = small.tile([128, 1], F32, name="recip")
            nc.vector.reciprocal(recip[:], o_ps[:, DV:DV + 1])
            o_sb = small.tile([128, DV], F32, name="o_sb")
            nc.vector.tensor_scalar_mul(o_sb[:], o_ps[:, 0:DV], recip[:])
            nc.sync.dma_start(out_ext[ti * 128:(ti + 1) * 128, :], o_sb[:])
            if ti + 6 < 2 * Q4:
                preload(ti + 6)

    nc.finalize()
    return nc


_CACHED = {}


def _get_kernel(mode: str):
    if mode not in _CACHED:
        _CACHED[mode] = build_kernel(mode)
    return _CACHED[mode]


def _prepare_in_maps(X, W_Q, W_K, W_V, mode):
    def wlayout(W):
        # w[p, dt, c] = W[dt*128 + p, c]
        n = W.shape[1]
        return np.ascontiguousarray(
            W.reshape(NT, 128, n).transpose(1, 0, 2)
        ).astype(ml_dtypes.bfloat16)

    wq = wlayout(W_Q)
    wk = wlayout(W_K)
    wv = wlayout(W_V)

    # Triangle masks for the diagonal 128x128 block of diagonal-band tiles.
    # Element [p, x]: key-in-block p, query-in-block x.
    # seg0 (own stripe): visible iff p <= x.
    # seg1 (remote stripe): s=0 cores: visible iff p < x; s=1: p <= x.
    p_idx = np.arange(128)[:, None]
    x_idx = np.arange(128)[None, :]
    tri_incl = np.where(p_idx <= x_idx, 0.0, MASK).astype(np.float32)
    tri_excl = np.where(p_idx < x_idx, 0.0, MASK).astype(np.float32)

    in_maps = []
    for core in range(8):
        b, s = core // 2, core % 2
        # column-half-major layout: xt[p, h, dt, c] = X[b, stripe row
        # h*512 + c, dt*128 + p]
        loc = X[b, s::2, :].reshape(LQ, NT, 128).transpose(2, 1, 0)
        xt = np.ascontiguousarray(
            np.stack([loc[:, :, 0:CHUNK], loc[:, :, CHUNK:LQ]], axis=1)
        ).astype(ml_dtypes.bfloat16)
        trimask = np.stack(
            [tri_incl, tri_excl if s == 0 else tri_incl], axis=1
        )  # [128, 2, 128]
        conds = np.zeros((1, 2), np.uint32)
        conds[0, 1 - s] = 1  # the pair peer's slot in the gather
        in_maps.append(
            {"XT": xt, "WQ": wq, "WK": wk, "WV": wv,
             "TRIMASK": np.ascontiguousarray(trimask), "CONDS": conds}
        )
    return in_maps


def _assemble(results):
    Z = np.empty((B, L, DV), np.float32)
    for core in range(8):
        b, s = core // 2, core % 2
        Z[b, s::2, :] = results[core]["OUT"]
    return Z


def kernel(X, W_Q, W_K, W_V):
    nc = _get_kernel(MODE)
    in_maps = _prepare_in_maps(X, W_Q, W_K, W_V, MODE)
    res = run_bass_kernel_spmd(nc, in_maps, core_ids=list(range(8)))
    return _assemble(res.results)
